# revision 9
# baseline (speedup 1.0000x reference)
"""Causal self-attention (B=2, T=2048, C=1024, 16 heads) on 8 TRN2 NeuronCores.

Sharding: 2-way data parallel (batch) x 4-way tensor parallel (heads).
Core c handles batch c//4 and heads [4*(c%4) .. 4*(c%4)+3].

Per-core pipeline (fp32 PSUM accumulation everywhere):
  - host pre-transposes x[b] -> xT [C, T] and splits it (and all weights)
    into exact fp8e4m3 hi+lo pairs (hi = rt(a), lo = rt(a - hi), so
    hi+lo ~ a to ~0.15%, better than bf16).
  - projections (qkv, proj) run as fp8 DoubleRow matmuls: each DR
    instruction contracts 2x128 at 0.5 cycles/col, and the product
    expands to 3 terms (hi*hi + lo*hi + hi*lo), i.e. 1.33x the bf16
    matmul rate at ~2x its accuracy.
  - q/k written bf16 to SBUF in transposed layout [j, T]; the S^T
    matmuls stay bf16 (fp8 scores would breach the error budget).
  - v written as fp8 hi+lo per (t-tile, head) [128, 65] with a
    ones-column (hi=1, lo=0) so the PV matmul also emits the softmax
    denominator.
  - attention: S^T tiles [kblock=128, w<=512] = K^T.T @ Q^T in bf16,
    kblocks processed in pairs sharing one exp; exp on ScalarE
    (scale 1/8, bias -3 so exp fits fp8e4m3 range; the e^-3 cancels in
    the normalization) writes p in fp8; causal masking is
    multiplicative post-exp on diagonal tiles.  PV runs as 2 fp8
    DoubleRow matmuls per kblock pair (v_hi then v_lo halves), 2x the
    bf16 rate with only p carrying fp8 error (softmax normalization
    cancels most of it).
  - y^T = O^T[0:64] * recip(O^T[64]) (GpSimd partition-broadcasts the
    reciprocal), split into fp8 hi+lo for the proj lhsT.
  - z_partial = y^T.T @ Wp_rows; AllReduce(add) over the 4 cores of the
    same batch, chunked to overlap with compute.

Self-contained: hardcodes shapes; only imports the system concourse stack.
"""

import contextlib

import numpy as np
import ml_dtypes

B, T, C = 2, 2048, 1024
NH = 16
HS = 64
NCORES = 8
HPC = 4          # heads per core
CPC = HPC * HS   # channels per core (256)
P = 128
QB = 512         # query block (free dim of S^T / O^T tiles)
NQ = T // QB     # 4 query blocks
NTT = T // P     # 16 t-tiles / kblocks
KC = C // P      # 8 contraction chunks
GROUPS = [[0, 1, 2, 3], [4, 5, 6, 7]]

_CACHE = {}

DEFAULT_CFG = dict(
    loop=1,          # repeat body (timing instrument)
    with_cc=True,    # AllReduce (False: plain DMA out, for TimelineSim)
    n_devices=NCORES,
    fp8_pv=True,     # PV matmuls in fp8 DoubleRow (p raw e4m3, v hi+lo)
    fp8_proj=True,   # qkv/proj matmuls as 3-term fp8 DoubleRow
    tail_split=True, # last query block's output chunked per t-tile
    interleave=1,    # heads processed together in attention (1 or 2)
    merged=True,     # weave qkv/proj filler units into attention emission
    ppool_bufs=8,
    zpool_bufs=4,
    rpool_bufs=4,
    s_bufs=2,
    o_bufs=2,
    mm_bufs=2,
    weave_bias=2.0,  # <1: fillers front-loaded in each round; >1: back-loaded
    qi_first=1,      # first query block processed (rotation: 1 -> 1,2,3,0)
    exp_bias=-3.0,   # exp(s/8 + bias): keeps p below fp8e4m3 max (240)
)


def _build_nc(cfg):
    import concourse.tile as tile
    import concourse.mybir as mybir
    from concourse import bacc

    f32 = mybir.dt.float32
    bf16 = mybir.dt.bfloat16
    f8 = mybir.dt.float8e4

    nc = bacc.Bacc(
        "TRN2",
        target_bir_lowering=False,
        debug=False,
        enable_asserts=True,
        num_devices=cfg["n_devices"],
    )
    aps = dict(
        bqk=nc.dram_tensor("bqk", [2 * CPC], f32, kind="ExternalInput").ap(),
        bv=nc.dram_tensor("bv", [CPC], f32, kind="ExternalInput").ap(),
        bp=nc.dram_tensor("bp", [C], f32, kind="ExternalInput").ap(),
        out=nc.dram_tensor("out", [T, C], bf16, kind="ExternalOutput").ap(),
    )
    if cfg["fp8_proj"]:
        for nm, shape in (("xT", [C, T]), ("wqk", [C, 2 * CPC]),
                          ("wv", [C, CPC]), ("wp", [CPC, C])):
            for s in ("hi", "lo"):
                aps[f"{nm}_{s}"] = nc.dram_tensor(
                    f"{nm}_{s}", shape, f8, kind="ExternalInput").ap()
    else:
        aps["xT"] = nc.dram_tensor("xT", [C, T], bf16, kind="ExternalInput").ap()
        aps["wqk"] = nc.dram_tensor("wqk", [C, 2 * CPC], bf16, kind="ExternalInput").ap()
        aps["wv"] = nc.dram_tensor("wv", [C, CPC], bf16, kind="ExternalInput").ap()
        aps["wp"] = nc.dram_tensor("wp", [CPC, C], bf16, kind="ExternalInput").ap()

    with tile.TileContext(nc) as tc, contextlib.ExitStack() as ctx:
        pools = dict(
            consts=ctx.enter_context(tc.tile_pool(name="consts", bufs=1)),
            big=ctx.enter_context(tc.tile_pool(name="big", bufs=1)),
            ppool=ctx.enter_context(tc.tile_pool(name="ppool", bufs=cfg["ppool_bufs"])),
            zpool=ctx.enter_context(tc.tile_pool(name="zpool", bufs=cfg["zpool_bufs"])),
            rpool=ctx.enter_context(tc.tile_pool(name="rpool", bufs=cfg["rpool_bufs"])),
            ps_mm=ctx.enter_context(tc.tile_pool(name="ps_mm", bufs=cfg["mm_bufs"], space="PSUM")),
            ps_s=ctx.enter_context(tc.tile_pool(name="ps_s", bufs=cfg["s_bufs"], space="PSUM")),
            ps_o=ctx.enter_context(tc.tile_pool(name="ps_o", bufs=cfg["o_bufs"], space="PSUM")),
            dram=ctx.enter_context(tc.tile_pool(name="dram", bufs=2, space="DRAM")),
        )
        state = _emit_consts(nc, mybir, aps, pools, cfg)
        for _rep in range(cfg["loop"]):
            _emit_body(nc, mybir, aps, pools, state, cfg)

    nc.compile()
    return nc


def _emit_consts(nc, mybir, aps, pools, cfg):
    f32 = mybir.dt.float32
    bf16 = mybir.dt.bfloat16
    f8 = mybir.dt.float8e4
    Alu = mybir.AluOpType
    consts, big = pools["consts"], pools["big"]
    st = {}

    # One DMA per tensor, spread across both HWDGE queues.
    if cfg["fp8_proj"]:
        for s in ("hi", "lo"):
            st[f"wv_{s}"] = consts.tile([P, KC, CPC], f8, name=f"wv_{s}")
            nc.sync.dma_start(st[f"wv_{s}"], aps[f"wv_{s}"].rearrange("(o p) m -> p o m", p=P))
        xr = {}
        for s in ("hi", "lo"):
            st[f"xT_{s}"] = big.tile([P, KC, T], f8, name=f"xT_{s}")
            xr[s] = aps[f"xT_{s}"].rearrange("(o p) t -> p o t", p=P)
            nc.sync.dma_start(st[f"xT_{s}"][:, :, 0:QB], xr[s][:, :, 0:QB])
        for s in ("hi", "lo"):
            st[f"wqk_{s}"] = consts.tile([P, KC, 2 * CPC], f8, name=f"wqk_{s}")
            nc.sync.dma_start(st[f"wqk_{s}"], aps[f"wqk_{s}"].rearrange("(o p) m -> p o m", p=P))
            st[f"wp_{s}"] = consts.tile([P, CPC // P, C], f8, name=f"wp_{s}")
            nc.sync.dma_start(st[f"wp_{s}"], aps[f"wp_{s}"].rearrange("(o p) m -> p o m", p=P))
    else:
        st["wv_sb"] = consts.tile([P, KC, CPC], bf16, name="wv_sb")
        nc.sync.dma_start(st["wv_sb"], aps["wv"].rearrange("(o p) m -> p o m", p=P))
        st["xT_sb"] = big.tile([P, KC, T], bf16, name="xT_sb")
        xT_r = aps["xT"].rearrange("(o p) t -> p o t", p=P)
        nc.sync.dma_start(st["xT_sb"][:, :, 0:QB], xT_r[:, :, 0:QB])
        st["wqk_sb"] = consts.tile([P, KC, 2 * CPC], bf16, name="wqk_sb")
        nc.sync.dma_start(st["wqk_sb"], aps["wqk"].rearrange("(o p) m -> p o m", p=P))
        st["wp_sb"] = consts.tile([P, CPC // P, C], bf16, name="wp_sb")
        nc.sync.dma_start(st["wp_sb"], aps["wp"].rearrange("(o p) m -> p o m", p=P))

    bqk_sb = consts.tile([P, 2 * CPC // P], f32)
    nc.sync.dma_start(bqk_sb, aps["bqk"].rearrange("(o p) -> p o", p=P))
    bv_row = consts.tile([1, CPC], f32)
    nc.sync.dma_start(bv_row, aps["bv"][None, :])
    bv_bc = consts.tile([P, CPC], f32)
    nc.gpsimd.partition_broadcast(bv_bc, bv_row)
    bp_row = consts.tile([1, C], f32)
    nc.sync.dma_start(bp_row, aps["bp"][None, :])
    bp_bc = consts.tile([P, C], f32)
    nc.gpsimd.partition_broadcast(bp_bc, bp_row)

    # multiplicative causal masks for the diagonal-block offsets:
    # masks[r, p, c] = 1.0 if c >= 128*p + r else 0.0   (c within the qblock)
    masks = consts.tile([P, 4, QB], bf16)
    nc.vector.memset(masks, 1.0)
    for pos in range(4):
        nc.gpsimd.affine_select(
            out=masks[:, pos, :],
            in_=masks[:, pos, :],
            pattern=[[1, QB]],
            compare_op=Alu.is_ge,
            fill=0.0,
            base=-P * pos,
            channel_multiplier=-1,
        )

    ebias = consts.tile([P, 1], f32, name="ebias")
    nc.vector.memset(ebias, cfg["exp_bias"])
    st["ebias"] = ebias

    # warm the exp table set (~2.7us load) while DMAs stream in
    warm = consts.tile([1, 1], f32)
    nc.vector.memset(warm, 0.0)
    warm2 = consts.tile([1, 1], f32)
    nc.scalar.activation(warm2, warm, mybir.ActivationFunctionType.Exp)

    qk_sb = big.tile([P, 4, T], bf16)   # mi 0-1: q heads, 2-3: k heads
    st.update(bqk_sb=bqk_sb, bv_bc=bv_bc, bp_bc=bp_bc, masks=masks, qk_sb=qk_sb)

    if cfg["fp8_proj"]:
        st["y_hi"] = big.tile([P, CPC // P, T], f8, name="y_hi")
        st["y_lo"] = big.tile([P, CPC // P, T], f8, name="y_lo")
    else:
        st["y_sb"] = big.tile([P, CPC // P, T], bf16, name="y_sb")

    if cfg["fp8_pv"]:
        # [.., s, h, 0:64]=v hi/lo, [.., s, h, 64]= 1.0 (hi) / 0.0 (lo)
        v8 = big.tile([P, NTT, 2, HPC, 66], f8)
        nc.vector.memset(v8[:, :, 0, :, 64:65], 1.0)
        nc.vector.memset(v8[:, :, 1, :, 64:65], 0.0)
        st["v8"] = v8
    else:
        v_sb = big.tile([P, NTT, HPC, 66], bf16)  # [.., 0:64]=v, [.., 64]=1.0
        nc.vector.memset(v_sb[:, :, :, 64:65], 1.0)
        st["v_sb"] = v_sb
    return st


def _emit_body(nc, mybir, aps, pools, st, cfg):
    f32 = mybir.dt.float32
    bf16 = mybir.dt.bfloat16
    f8 = mybir.dt.float8e4
    Alu = mybir.AluOpType
    Act = mybir.ActivationFunctionType
    DR = mybir.MatmulPerfMode.DoubleRow
    ppool, zpool, rpool = pools["ppool"], pools["zpool"], pools["rpool"]
    ps_mm, ps_s, ps_o, dram = pools["ps_mm"], pools["ps_s"], pools["ps_o"], pools["dram"]
    bqk_sb, bv_bc, bp_bc, masks = st["bqk_sb"], st["bv_bc"], st["bp_bc"], st["masks"]
    qk_sb = st["qk_sb"]
    out = aps["out"]
    fp8_proj, fp8_pv = cfg["fp8_proj"], cfg["fp8_pv"]

    if fp8_proj:
        xr = {s: aps[f"xT_{s}"].rearrange("(o p) t -> p o t", p=P) for s in ("hi", "lo")}
        # 3-term split product: hi*hi + lo_x*hi_w + hi_x*lo_w
        qkv_terms = [("hi", "hi"), ("lo", "hi"), ("hi", "lo")]
    else:
        xT_r = aps["xT"].rearrange("(o p) t -> p o t", p=P)

    # ---------- emission units ----------
    def xdma_unit(tc_i):
        def emit():
            tsl = slice(tc_i * QB, (tc_i + 1) * QB)
            if fp8_proj:
                for s in ("hi", "lo"):
                    nc.sync.dma_start(st[f"xT_{s}"][:, :, tsl], xr[s][:, :, tsl])
            else:
                nc.sync.dma_start(st["xT_sb"][:, :, tsl], xT_r[:, :, tsl])
        return emit

    def qk_unit(tc_i, mi):
        def emit():
            tsl = slice(tc_i * QB, (tc_i + 1) * QB)
            ps_qk = ps_mm.tile([P, QB], f32, tag="mm", name="ps_qk")
            if fp8_proj:
                nmm = len(qkv_terms) * (KC // 2)
                i = 0
                for ci in range(KC // 2):
                    for sx, sw in qkv_terms:
                        nc.tensor.matmul(
                            ps_qk,
                            st[f"wqk_{sw}"][:, 2 * ci : 2 * ci + 2, mi * P : (mi + 1) * P],
                            st[f"xT_{sx}"][:, 2 * ci : 2 * ci + 2, tsl],
                            start=(i == 0),
                            stop=(i == nmm - 1),
                            perf_mode=DR,
                        )
                        i += 1
            else:
                for ci in range(KC):
                    nc.tensor.matmul(
                        ps_qk,
                        st["wqk_sb"][:, ci, mi * P : (mi + 1) * P],
                        st["xT_sb"][:, ci, tsl],
                        start=(ci == 0),
                        stop=(ci == KC - 1),
                    )
            if fp8_proj:
                nc.vector.tensor_scalar(
                    qk_sb[:, mi, tsl], ps_qk, 1.0 / 32.0,
                    bqk_sb[:, mi : mi + 1], Alu.mult, Alu.add,
                )
            else:
                nc.vector.tensor_scalar_add(
                    qk_sb[:, mi, tsl], ps_qk, bqk_sb[:, mi : mi + 1]
                )
        return emit

    def v_unit(tt):
        def emit():
            ps_v = ps_mm.tile([P, CPC], f32, tag="mm", name="ps_v")
            if fp8_proj:
                nmm = len(qkv_terms) * (KC // 2)
                i = 0
                for ci in range(KC // 2):
                    for sx, sw in qkv_terms:
                        nc.tensor.matmul(
                            ps_v,
                            st[f"xT_{sx}"][:, 2 * ci : 2 * ci + 2, tt * P : (tt + 1) * P],
                            st[f"wv_{sw}"][:, 2 * ci : 2 * ci + 2, :],
                            start=(i == 0),
                            stop=(i == nmm - 1),
                            perf_mode=DR,
                        )
                        i += 1
            else:
                for ci in range(KC):
                    nc.tensor.matmul(
                        ps_v,
                        st["xT_sb"][:, ci, tt * P : (tt + 1) * P],
                        st["wv_sb"][:, ci, :],
                        start=(ci == 0),
                        stop=(ci == KC - 1),
                    )
            if fp8_pv:
                vtmp = rpool.tile([P, CPC], bf16, tag="vt", name="vtmp")
                if fp8_proj:
                    nc.vector.scalar_tensor_tensor(
                        vtmp, ps_v, 1.0 / 32.0, bv_bc, Alu.mult, Alu.add
                    )
                else:
                    nc.vector.tensor_tensor(vtmp, ps_v, bv_bc, Alu.add)
                for h in range(HPC):
                    hsl = slice(h * HS, (h + 1) * HS)
                    nc.vector.tensor_copy(st["v8"][:, tt, 0, h, 0:64], vtmp[:, hsl])
                    nc.vector.tensor_tensor(
                        st["v8"][:, tt, 1, h, 0:64], vtmp[:, hsl],
                        st["v8"][:, tt, 0, h, 0:64], Alu.subtract,
                    )
            else:
                for h in range(HPC):
                    if fp8_proj:
                        nc.vector.scalar_tensor_tensor(
                            st["v_sb"][:, tt, h, 0:64],
                            ps_v[:, h * HS : (h + 1) * HS], 1.0 / 32.0,
                            bv_bc[:, h * HS : (h + 1) * HS],
                            Alu.mult, Alu.add,
                        )
                    else:
                        nc.vector.tensor_tensor(
                            st["v_sb"][:, tt, h, 0:64],
                            ps_v[:, h * HS : (h + 1) * HS],
                            bv_bc[:, h * HS : (h + 1) * HS],
                            Alu.add,
                        )
        return emit

    z_ts = {}

    def proj_unit(qi, tl, n, z_loc, split):
        def emit():
            tt = qi * 4 + tl
            ps_z = ps_mm.tile([P, QB], f32, tag="mm", name="ps_z")
            if fp8_proj:
                terms = [("hi", "hi"), ("lo", "hi"), ("hi", "lo")]
                for t, (sy, sw) in enumerate(terms):
                    nc.tensor.matmul(
                        ps_z,
                        st[f"y_{sy}"][:, 0:2, tt * P : (tt + 1) * P],
                        st[f"wp_{sw}"][:, 0:2, n * QB : (n + 1) * QB],
                        start=(t == 0),
                        stop=(t == len(terms) - 1),
                        perf_mode=DR,
                    )
            else:
                for kc2 in range(CPC // P):
                    nc.tensor.matmul(
                        ps_z,
                        st["y_sb"][:, kc2, tt * P : (tt + 1) * P],
                        st["wp_sb"][:, kc2, n * QB : (n + 1) * QB],
                        start=(kc2 == 0),
                        stop=(kc2 == CPC // P - 1),
                    )
            if n == 0:
                z_ts[tt] = zpool.tile([P, C], bf16, tag="z", name="z_t")
            z_t = z_ts[tt]
            if fp8_proj:
                nc.vector.scalar_tensor_tensor(
                    z_t[:, n * QB : (n + 1) * QB], ps_z, 1.0 / 32.0,
                    bp_bc[:, n * QB : (n + 1) * QB], Alu.mult, Alu.add,
                )
            else:
                nc.vector.tensor_tensor(
                    z_t[:, n * QB : (n + 1) * QB], ps_z,
                    bp_bc[:, n * QB : (n + 1) * QB], Alu.add
                )
            if n == 1:
                nc.sync.dma_start(z_loc[tl * P : (tl + 1) * P, :], z_t)
                del z_ts[tt]
                if split and tl % 2 == 1:
                    _reduce_out(nc, mybir, pools, cfg,
                                z_loc[(tl - 1) * P : (tl + 1) * P, :],
                                out[(tt - 1) * P : (tt + 1) * P, :], 2 * P)
        return emit

    def attn_groups_fp8(qi):
        """Kblock pairs; S^T bf16 at pair width, exp->fp8, PV fp8 DR."""
        qsl = slice(qi * QB, (qi + 1) * QB)
        kmax = 4 * qi + 4
        npair = kmax // 2
        # pair j covers kblocks (2j, 2j+1) at the even kblock's live width
        pairs = [(j, QB - P * max(0, 2 * j - 4 * qi)) for j in range(npair)]
        o_ts = {}
        v8 = st["v8"]

        def head_pair(h, j, w_p):
            def emit():
                po = 64 * (h % 2)
                kT = qk_sb[po : po + 64, 2 + h // 2, :]
                qT = qk_sb[po : po + 64, h // 2, qsl]
                if h not in o_ts:
                    o_ts[h] = ps_o.tile([65, QB], f32, tag="o", name="o_t")
                o_t = o_ts[h]
                s_t = ps_s.tile([P, 2, QB], f32, tag="s", name="s_t")
                for i in range(2):
                    kb = 2 * j + i
                    nc.tensor.matmul(
                        s_t[:, i, 0:w_p],
                        kT[:, kb * P : (kb + 1) * P],
                        qT[:, QB - w_p :],
                        start=True,
                        stop=True,
                    )
                p_t = ppool.tile([P, 2, QB], f8, tag="p2", name="p_t")
                nc.scalar.activation(
                    p_t[:, 0:2, 0:w_p], s_t[:, 0:2, 0:w_p], Act.Exp,
                    scale=0.125, bias=st["ebias"],
                )
                for i in range(2):
                    pos = 2 * j + i - 4 * qi
                    if pos >= 0:
                        nc.vector.tensor_tensor(
                            p_t[:, i, 0:w_p],
                            p_t[:, i, 0:w_p],
                            masks[:, pos, QB - w_p :],
                            Alu.mult,
                        )
                for s_i in range(2):  # v hi, v lo halves
                    nc.tensor.matmul(
                        o_t[:, QB - w_p :],
                        v8[:, 2 * j : 2 * j + 2, s_i, h, 0:65],
                        p_t[:, 0:2, 0:w_p],
                        start=(j == 0 and s_i == 0),
                        stop=(j == npair - 1 and s_i == 1),
                        perf_mode=DR,
                    )
            return emit

        def finisher(h):
            def emit():
                po = 64 * (h % 2)
                r_t = rpool.tile([1, QB], f32, tag="r", name="r_t")
                nc.vector.reciprocal(r_t, o_ts[h][64:65, :])
                rb_t = rpool.tile([64, QB], f32, tag="rb", name="rb_t")
                nc.gpsimd.partition_broadcast(rb_t, r_t)
                if fp8_proj:
                    y32 = rpool.tile([P, QB], f32, tag="y32", name="y32")
                    nc.vector.tensor_tensor(
                        y32[po : po + 64, :], o_ts[h][0:64, :], rb_t, Alu.mult
                    )
                    nc.vector.tensor_copy(
                        st["y_hi"][po : po + 64, h // 2, qsl], y32[po : po + 64, :]
                    )
                    nc.vector.tensor_tensor(
                        st["y_lo"][po : po + 64, h // 2, qsl],
                        y32[po : po + 64, :],
                        st["y_hi"][po : po + 64, h // 2, qsl],
                        Alu.subtract,
                    )
                else:
                    nc.vector.tensor_tensor(
                        st["y_sb"][po : po + 64, h // 2, qsl], o_ts[h][0:64, :],
                        rb_t, Alu.mult,
                    )
                del o_ts[h]
            return emit

        il = cfg["interleave"]
        units = []
        for hp in range(HPC // il):
            heads = tuple(range(il * hp, il * hp + il))
            for j, w_p in pairs:
                for h in heads:
                    units.append(head_pair(h, j, w_p))
            for h in heads:
                units.append(finisher(h))
        return units

    def attn_groups_bf16(qi):
        """Original bf16 path: live-width tiles, paired exp, bf16 PV."""
        qsl = slice(qi * QB, (qi + 1) * QB)
        kmax = 4 * qi + 4
        groups = []
        kb = 0
        while kb < kmax:
            if kb + 1 < kmax:
                w0 = QB - P * max(0, kb - 4 * qi)
                w1 = QB - P * max(0, kb + 1 - 4 * qi)
                groups.append([(kb, 0, w0), (kb + 1, w0, w1)])
                kb += 2
                continue
            w0 = QB - P * max(0, kb - 4 * qi)
            groups.append([(kb, 0, w0)])
            kb += 1

        o_ts = {}
        v_sb = st["v_sb"]

        def head_group(h, subs):
            def emit():
                po = 64 * (h % 2)
                kT = qk_sb[po : po + 64, 2 + h // 2, :]
                qT = qk_sb[po : po + 64, h // 2, qsl]
                if h not in o_ts:
                    o_ts[h] = ps_o.tile([65, QB], f32, tag="o", name="o_t")
                o_t = o_ts[h]
                tot = subs[-1][1] + subs[-1][2]
                s_t = ps_s.tile([P, 2 * QB], f32, tag="s", name="s_t")
                for kb, off, w in subs:
                    nc.tensor.matmul(
                        s_t[:, off : off + w],
                        kT[:, kb * P : (kb + 1) * P],
                        qT[:, QB - w :],
                        start=True,
                        stop=True,
                    )
                p_t = ppool.tile([P, 2 * QB], bf16, tag="p2", name="p_t")
                nc.scalar.activation(
                    p_t[:, :tot], s_t[:, :tot], Act.Exp, scale=0.125
                )
                for kb, off, w in subs:
                    pos = kb - 4 * qi
                    if pos >= 0:
                        nc.vector.tensor_tensor(
                            p_t[:, off : off + w],
                            p_t[:, off : off + w],
                            masks[:, pos, P * pos :],
                            Alu.mult,
                        )
                for kb, off, w in subs:
                    nc.tensor.matmul(
                        o_t[:, QB - w :],
                        v_sb[:, kb, h, 0:65],
                        p_t[:, off : off + w],
                        start=(kb == 0),
                        stop=(kb == kmax - 1),
                    )
            return emit

        def finisher(h):
            def emit():
                po = 64 * (h % 2)
                r_t = rpool.tile([1, QB], f32, tag="r", name="r_t")
                nc.vector.reciprocal(r_t, o_ts[h][64:65, :])
                rb_t = rpool.tile([64, QB], f32, tag="rb", name="rb_t")
                nc.gpsimd.partition_broadcast(rb_t, r_t)
                if fp8_proj:
                    y32 = rpool.tile([P, QB], f32, tag="y32", name="y32")
                    nc.vector.tensor_tensor(
                        y32[po : po + 64, :], o_ts[h][0:64, :], rb_t, Alu.mult
                    )
                    nc.vector.tensor_copy(
                        st["y_hi"][po : po + 64, h // 2, qsl], y32[po : po + 64, :]
                    )
                    nc.vector.tensor_tensor(
                        st["y_lo"][po : po + 64, h // 2, qsl],
                        y32[po : po + 64, :],
                        st["y_hi"][po : po + 64, h // 2, qsl],
                        Alu.subtract,
                    )
                else:
                    nc.vector.tensor_tensor(
                        st["y_sb"][po : po + 64, h // 2, qsl], o_ts[h][0:64, :], rb_t,
                        Alu.mult,
                    )
                del o_ts[h]
            return emit

        il = cfg["interleave"]
        units = []
        for hp in range(HPC // il):
            heads = tuple(range(il * hp, il * hp + il))
            for subs in groups:
                for h in heads:
                    units.append(head_group(h, subs))
            for h in heads:
                units.append(finisher(h))
        return units

    attn_groups = attn_groups_fp8 if fp8_pv else attn_groups_bf16

    # ---------- merged schedule ----------
    def qkv_units(tc_i):
        u = []
        if tc_i > 0:
            u.append(xdma_unit(tc_i))
        vu = [v_unit(tc_i * 4 + tl) for tl in range(4)]
        qu = [qk_unit(tc_i, mi) for mi in range(4)]
        # chunk 0: v first -- wv + xT chunk 0 are the first DMAs to land
        return u + (vu + qu if tc_i == 0 else qu + vu)

    z_locs = {}

    def proj_units(qi, split):
        z_locs[qi] = dram.tile([QB, C], bf16, tag="zloc", name="z_loc")
        u = []
        for tl in range(4):
            for n in range(2):
                u.append(proj_unit(qi, tl, n, z_locs[qi], split))
        return u

    def finish_block(qi, split):
        if not split:
            def emit():
                _reduce_out(nc, mybir, pools, cfg, z_locs[qi],
                            out[qi * QB : (qi + 1) * QB, :], QB)
            return [emit]
        return []

    if not cfg["merged"]:
        for tc_i in range(NQ):
            for u in qkv_units(tc_i):
                u()
        for qi_idx, qi in enumerate(
            [(cfg["qi_first"] + i) % NQ for i in range(NQ)]
        ):
            split = cfg["tail_split"] and qi_idx == NQ - 1
            for u in attn_groups(qi):
                u()
            for u in proj_units(qi, split) + finish_block(qi, split):
                u()
        return

    # merged: xT DMAs up front, then qi rounds with fillers woven in
    for u in qkv_units(0):
        u()
    for qi in range(NQ):
        split = cfg["tail_split"] and qi == NQ - 1
        att = attn_groups(qi)
        fillers = []
        if qi + 1 < NQ:
            fillers += qkv_units(qi + 1)
        if qi > 0:
            fillers += proj_units(qi - 1, False) + finish_block(qi - 1, False)
        # weave fillers evenly among attention groups
        n_att, n_fill = len(att), len(fillers)
        fi = 0
        bias = cfg["weave_bias"]
        for gi, u in enumerate(att):
            u()
            want = int((((gi + 1) / n_att) ** bias) * n_fill)
            while fi < want:
                fillers[fi]()
                fi += 1
        while fi < n_fill:
            fillers[fi]()
            fi += 1
    for u in proj_units(NQ - 1, split) + finish_block(NQ - 1, split):
        u()


def _reduce_out(nc, mybir, pools, cfg, z_loc_ap, out_ap, rows):
    f32 = mybir.dt.float32
    Alu = mybir.AluOpType
    bf16 = mybir.dt.bfloat16
    if cfg["with_cc"]:
        z_red = pools["dram"].tile([rows, C], bf16, tag=f"zred{rows}")
        nc.gpsimd.collective_compute(
            "AllReduce",
            Alu.add,
            replica_groups=GROUPS,
            ins=[z_loc_ap.opt()],
            outs=[z_red.opt()],
        )
        nc.sync.dma_start(out_ap, z_red)
    else:
        nc.sync.dma_start(out_ap, z_loc_ap)


def get_nc(**overrides):
    cfg = dict(DEFAULT_CFG)
    cfg.update(overrides)
    key = tuple(sorted(cfg.items()))
    if key not in _CACHE:
        _CACHE[key] = _build_nc(cfg)
    return _CACHE[key]


def _split8(a):
    hi = a.astype(ml_dtypes.float8_e4m3)
    lo = (a - hi.astype(np.float32)).astype(ml_dtypes.float8_e4m3)
    return np.ascontiguousarray(hi), np.ascontiguousarray(lo)


def make_in_maps(x, w_attn, b_attn, w_proj, b_proj, cfg=None):
    cfg = cfg or DEFAULT_CFG
    x = np.asarray(x, dtype=np.float32)
    w_attn = np.asarray(w_attn, dtype=np.float32)
    b_attn = np.asarray(b_attn, dtype=np.float32)
    w_proj = np.asarray(w_proj, dtype=np.float32)
    b_proj = np.asarray(b_proj, dtype=np.float32)
    bf = ml_dtypes.bfloat16

    in_maps = []
    for core in range(NCORES):
        b, g = core // 4, core % 4
        hsl = slice(g * CPC, (g + 1) * CPC)
        wq = w_attn[:, 0:C][:, hsl]
        wk = w_attn[:, C : 2 * C][:, hsl]
        wv_ = w_attn[:, 2 * C : 3 * C][:, hsl]
        m = {
            "bqk": np.concatenate(
                [b_attn[0:C][hsl], b_attn[C : 2 * C][hsl]]
            ).astype(np.float32),
            "bv": np.ascontiguousarray(b_attn[2 * C : 3 * C][hsl]).astype(np.float32),
            # every core in a reduce group adds its bp share pre-AllReduce
            "bp": (b_proj / 4.0).astype(np.float32),
        }
        xT = np.ascontiguousarray(x[b].T)
        wqk = np.ascontiguousarray(np.concatenate([wq, wk], axis=1))
        wv_c = np.ascontiguousarray(wv_)
        wp_c = np.ascontiguousarray(w_proj[hsl, :])
        if cfg["fp8_proj"]:
            # weights sit near e4m3's subnormal range (sigma=1/32); scale by
            # 32 so hi+lo splits stay exact, un-scaled in the bias-add
            for nm, arr in (("xT", xT), ("wqk", wqk * 32.0), ("wv", wv_c * 32.0),
                            ("wp", wp_c * 32.0)):
                m[f"{nm}_hi"], m[f"{nm}_lo"] = _split8(arr)
        else:
            m["xT"] = xT.astype(bf)
            m["wqk"] = wqk.astype(bf)
            m["wv"] = wv_c.astype(bf)
            m["wp"] = wp_c.astype(bf)
        in_maps.append(m)
    return in_maps


def kernel(x, w_attn, b_attn, w_proj, b_proj):
    from concourse.bass_utils import run_bass_kernel_spmd

    nc = get_nc()
    in_maps = make_in_maps(x, w_attn, b_attn, w_proj, b_proj)
    res = run_bass_kernel_spmd(nc, in_maps, core_ids=list(range(NCORES))).results
    out = np.empty((B, T, C), np.float32)
    out[0] = res[0]["out"].astype(np.float32)
    out[1] = res[4]["out"].astype(np.float32)
    return out
